# revision 22
# baseline (speedup 1.0000x reference)
"""Multi-head attention (B=4, S=1024, H=1024, 16 heads) on 8 TRN2 NeuronCores.

Sharding: batch x head-group - each core owns (one batch, 8 heads) as
cores = 4 batches x 2 head-groups.  x^T for the core's batch is loaded once
(2.1 MB) and stays resident; projections run per head-pair (128 j columns).

Scores are computed in transposed [t, s] layout with ROW-TILED matmuls:
each head contracts over only its own 64 features, so the two heads of a
pair run as two concurrent K=64 matmuls on disjoint PE row halves
(tile_position (0,0) / (64,0)) writing separate PSUM tiles.  This removes
the 2x zero-padding waste of K=128 padded scores and the kt pad memsets.

Softmax bias handling uses exp(s + b) = exp(s) * exp(b): the host
precomputes exp(bias^T) in bf16, the scalar engine computes exp(scores)
straight out of PSUM (its only job), and the vector engine multiplies by
the bias factor as a bf16*bf16 SBUF op (2x DVE mode).  The softmax
denominator rides along as a ones-column appended to V; normalization
happens on the host.

The emission schedule is software-pipelined over 32 pair-slots (pair, tt):
projections for pair p+1 interleave into pair p's attention slots, PV for
pair p-1 runs during pair p's score slots.  Steady state alternates
between ACT (2 exps ~2.1us/slot) and PE as the gate; PE fills its slack
with projection/PV work.

PSUM budget (8 banks): scores 3x[128,1024] (6) - the third buffer means a
slot's score matmuls are gated by exps from 1.5 slots back (long retired),
keeping the ACT pipeline fed and letting the row-tiled head pairs
co-issue; one shared 2-buf pool of one-bank tiles (2) serves q/k chain
accumulators, v tt-groups, and PV accumulator groups of two s-chunks.

All DRAM operands are host-relaid so every DMA packet is a 2-4 KiB
contiguous run per partition.  Compute dtype bf16 (f32 PSUM); masks fold
into the bias on host as -1e30 (exp == 0) if ever set.
"""

import numpy as np
import ml_dtypes

NUM_HEAD = 16
B, S, H = 4, 1024, 1024
HD = H // NUM_HEAD            # 64
N_CORES = 8
KO = H // 128                 # 8 contraction chunks
SC = S // 512                 # 2 (N=512 matmul chunks)
TT = S // 128                 # 8 (128-row tiles over s or t)
NP = 4                        # head-pairs per core
HPC = 2                       # heads per pair
NHEADS = NP * HPC             # 8 heads per core

BF16 = ml_dtypes.bfloat16

_CACHE = {}

# Set by test harness to capture profiling info.
TRACE = False
LAST_RESULTS = None


def _build_bass():
    from concourse import bacc
    import concourse.tile as tile
    import concourse.mybir as mybir
    from contextlib import ExitStack

    bf16 = mybir.dt.bfloat16
    f32 = mybir.dt.float32
    Exp = mybir.ActivationFunctionType.Exp
    Mult = mybir.AluOpType.mult

    nc = bacc.Bacc("TRN2", target_bir_lowering=False, debug=False)

    # host-relaid operands: per-partition lines are contiguous DRAM runs
    xt = nc.dram_tensor("xt", [128, KO, S], bf16, kind="ExternalInput")
    wq = nc.dram_tensor("wq", [128, NP, KO, 128], bf16, kind="ExternalInput")
    wk = nc.dram_tensor("wk", [128, NP, KO, 128], bf16, kind="ExternalInput")
    wv = nc.dram_tensor("wv", [128, NP, KO, 128], bf16, kind="ExternalInput")
    expb = nc.dram_tensor("expb", [NHEADS, 128, TT, S], bf16, kind="ExternalInput")
    out = nc.dram_tensor(
        "out", [NHEADS, 128, TT * (HD + 1)], bf16, kind="ExternalOutput"
    )

    with tile.TileContext(nc) as tc, ExitStack() as ctx:
        singles = ctx.enter_context(tc.tile_pool(name="singles", bufs=1))
        qtpool = ctx.enter_context(tc.tile_pool(name="qt", bufs=2))
        ktpool = ctx.enter_context(tc.tile_pool(name="kt", bufs=2))
        vpool = ctx.enter_context(tc.tile_pool(name="v", bufs=3))
        ptpool = ctx.enter_context(tc.tile_pool(name="pt", bufs=4))
        ebpool = ctx.enter_context(tc.tile_pool(name="eb", bufs=10))
        obhpool = ctx.enter_context(tc.tile_pool(name="obh", bufs=3))
        # shared one-bank pool for q/k chain accumulators, v tt-groups and
        # PV accumulator groups (2 banks total)
        ps_proj = ctx.enter_context(tc.tile_pool(name="ps_proj", bufs=2, space="PSUM"))
        # 3 score buffers (6 banks): a slot's score matmuls are then gated
        # by exps from 1.5 slots back (long retired), so the ACT pipeline
        # never waits on PE and the row-tiled head pairs can co-issue.
        ps_scores = ctx.enter_context(
            tc.tile_pool(name="ps_scores", bufs=3, space="PSUM")
        )

        # PE warm-up: a dead-weight matmul chain on (uninitialized) SBUF
        # starts the moment the PE comes up, covering the p-state ramp while
        # the first DMAs land.  The result is discarded.
        warm = singles.tile([128, 512], bf16, tag="warm")
        wps = ps_proj.tile([128, 512], f32, tag="ps_proj")
        NWARM = 26  # sized to cover the ~10us xt DMA latency at cold clock
        for i in range(NWARM):
            nc.tensor.matmul(
                wps[:], warm[:, 0:128], warm[:], start=(i == 0), stop=(i == NWARM - 1)
            )
        nc.vector.tensor_copy(out=warm[:, 0:1], in_=wps[:, 0:1])

        # Startup-critical loads: pair 0's weights and xt chunks first.
        w_sb = {}
        w_dram = {}
        for name, dram in (("q", wq), ("k", wk), ("v", wv)):
            w_sb[name] = singles.tile(
                [128, NP, KO, 128], bf16, tag=f"w{name}", name=f"w{name}_sb"
            )
            w_dram[name] = dram
        nc.sync.dma_start(out=w_sb["q"][:, 0, :, :], in_=wq[:, 0, :, :])
        nc.sync.dma_start(out=w_sb["k"][:, 0, :, :], in_=wk[:, 0, :, :])

        xt_sb = singles.tile([128, KO, S], bf16, tag="xt", name="xt_sb")
        # chunk-pair loads: 4 KiB contiguous per partition per transfer,
        # split into partition halves for transfer concurrency.
        for kc in range(0, KO, 2):
            nc.sync.dma_start(
                out=xt_sb[0:64, kc : kc + 2, :], in_=xt[0:64, kc : kc + 2, :]
            )
            nc.sync.dma_start(
                out=xt_sb[64:128, kc : kc + 2, :], in_=xt[64:128, kc : kc + 2, :]
            )

        qt_t = {}
        kt_t = {}
        vext_t = {}

        def gen_proj(p):
            """Yield (cols, fn) ops emitting head-pair p's q/k projections.

            Chains allocate one-bank [128,512] psum tiles from the shared
            2-buf pool; sc0/sc1 chains of the same proj pipeline through the
            two buffers, and the psum->sbuf cast rides with the last MM.
            """
            if p > 0:
                def dma_w(p=p):
                    for name in ("q", "k", "v"):
                        nc.sync.dma_start(
                            out=w_sb[name][:, p, :, :],
                            in_=w_dram[name][:, p, :, :],
                        )
                yield (0, dma_w)

            def alloc(p=p):
                qt_t[p] = qtpool.tile([128, S], bf16, tag="qt", name=f"qt{p}")
                kt_t[p] = ktpool.tile([128, S], bf16, tag="kt", name=f"kt{p}")
            yield (0, alloc)

            # q then k: [j on partitions, s free]; per 512-chunk a psum chain
            # of 8 K-accumulation matmuls, cast to bf16 SBUF when done.
            for name in ("q", "k"):
                for sc in range(SC):
                    ssl = slice(sc * 512, (sc + 1) * 512)
                    chain = {}
                    for kop in range(0, KO, 2):
                        def mm_pair(
                            p=p, name=name, ssl=ssl, kop=kop, chain=chain
                        ):
                            if kop == 0:
                                chain["ps"] = ps_proj.tile(
                                    [128, 512], f32,
                                    tag="ps_proj", name="ps_projc",
                                )
                            ps = chain["ps"]
                            for ko in (kop, kop + 1):
                                nc.tensor.matmul(
                                    ps[:],
                                    w_sb[name][:, p, ko, :],
                                    xt_sb[:, ko, ssl],
                                    start=(ko == 0),
                                    stop=(ko == KO - 1),
                                )
                            if kop + 2 == KO:
                                dst = qt_t[p] if name == "q" else kt_t[p]
                                nc.vector.tensor_copy(
                                    out=dst[:, ssl], in_=ps[:]
                                )
                        yield (1024, mm_pair)

        def gen_v(p):
            # v: [t on partitions, d free]; tt-chains grouped 4 per psum
            # bank, one combined cast per group into the vext slots.
            if p == 0:
                def dma_wv0():
                    nc.sync.dma_start(
                        out=w_sb["v"][:, 0, :, :], in_=wv[:, 0, :, :]
                    )
                yield (0, dma_wv0)

            def alloc_v(p=p):
                vext_t[p] = vpool.tile(
                    [128, HPC, TT, HD + 1], bf16, tag="vext", name=f"vext{p}"
                )
                nc.gpsimd.memset(vext_t[p][:, :, :, HD : HD + 1], 1.0)
            yield (0, alloc_v)
            group = {}
            for tt in range(TT):
                tsl = slice(tt * 128, (tt + 1) * 128)
                for kop in range(0, KO, 2):
                    def mm_pair_v(p=p, tsl=tsl, tt=tt, kop=kop, group=group):
                        if tt % 4 == 0 and kop == 0:
                            group["ps"] = ps_proj.tile(
                                [128, 4, HPC, HD], f32,
                                tag="ps_proj", name="ps_projv",
                            )
                        ps = group["ps"]
                        for ko in (kop, kop + 1):
                            nc.tensor.matmul(
                                ps[:, tt % 4, :, :],
                                xt_sb[:, ko, tsl],
                                w_sb["v"][:, p, ko, :],
                                start=(ko == 0),
                                stop=(ko == KO - 1),
                            )
                        if kop + 2 == KO and tt % 4 == 3:
                            # cast the 4-tt group; AP traversal order matches
                            # psum (tt, hpc, d) via strided out dims
                            g0 = tt - 3
                            nc.vector.tensor_copy(
                                out=vext_t[p][:, :, g0 : g0 + 4, 0:HD]
                                .transpose([0, 2, 1, 3]),
                                in_=ps[:, :, :, :],
                            )
                    yield (256, mm_pair_v)

        # Drain upfront what the first score tiles need.  q-sc0 and k-sc0
        # are interleaved chunk-wise (both advance while xt streams in) so
        # the first score matmuls (which need only the sc0 halves of qt/kt)
        # unblock one full chain earlier; q-sc1 (needed by the first exp)
        # follows over the already-resident chunks.
        ops0 = list(gen_proj(0))
        # ops0: [alloc, q-sc0 x4, q-sc1 x4, k-sc0 x4, k-sc1 x4]
        order = [ops0[0]]
        for i in range(4):
            order.append(ops0[1 + i])
            order.append(ops0[9 + i])
        order.extend(ops0[5:9])
        for _cols, fn in order:
            fn()
        leftover0 = ops0[13:]

        TOTAL_SLOTS = NP * TT  # 32 pair-slots
        eb_tiles = {}
        eb_cursor = 0

        def emit_eb_upto(limit):
            # eb tile index order: (pair, tt, h) = consumption order
            nonlocal eb_cursor
            while eb_cursor < min(limit, TOTAL_SLOTS * HPC):
                ps_, rem = divmod(eb_cursor, TT * HPC)
                tt2, h2 = divmod(rem, HPC)
                gi2 = ps_ * HPC + h2
                ebt = ebpool.tile([128, S], bf16, tag="eb", name=f"eb{eb_cursor}")
                nc.sync.dma_start(out=ebt[:], in_=expb[gi2, :, tt2, :])
                eb_tiles[(gi2, tt2)] = ebt
                eb_cursor += 1

        pt_t = {}
        obh_t = {}
        pso_cur = {}

        def emit_pv_chunk(gi_prev, sc8):
            hp = gi_prev % HPC
            if sc8 == 0:
                obh_t[gi_prev] = obhpool.tile(
                    [128, TT, HD + 1], bf16, tag="obh", name=f"obh{gi_prev}"
                )
            if hp == 0 and sc8 % 2 == 0:
                pso_cur[0] = ps_proj.tile(
                    [128, HPC, 2, HD + 1], f32, tag="ps_proj", name="pso_g"
                )
            pso = pso_cur[0]
            ssl = slice(sc8 * 128, (sc8 + 1) * 128)
            ptp = pt_t[gi_prev]
            vxp = vext_t[gi_prev // HPC]
            for ttp in range(TT):
                nc.tensor.matmul(
                    pso[:, hp, sc8 % 2, :],
                    ptp[:, ttp, ssl],
                    vxp[:, hp, ttp, :],
                    start=(ttp == 0),
                    stop=(ttp == TT - 1),
                )
            if sc8 % 2 == 1:
                nc.vector.tensor_copy(
                    out=obh_t[gi_prev][:, sc8 - 1 : sc8 + 1, :],
                    in_=pso[:, hp, :, :],
                )
                if sc8 % 4 == 3:
                    half = slice((sc8 - 3) * (HD + 1), (sc8 + 1) * (HD + 1))
                    nc.sync.dma_start(
                        out=out[gi_prev][:, half],
                        in_=obh_t[gi_prev][:, sc8 - 3 : sc8 + 1, :],
                    )

        qk_gen = None
        qk_total = qk_done = 0
        v_gen = None
        v_total = v_done = 0
        v_start = 0
        v_next = 0

        for p in range(NP):
            gi0, gi1 = p * HPC, p * HPC + 1
            # qk stream for pair p+1 over this pair's 8 slots (pair 0's
            # leftover k-sc1 rides at the front of pair 0's window)
            if p + 1 < NP:
                ops = (leftover0 if p == 0 else []) + list(gen_proj(p + 1))
                qk_gen = iter(ops)
                qk_total = sum(c for c, _ in ops)
                qk_done = 0
            else:
                qk_gen = iter(leftover0) if p == 0 else None
                qk_total = sum(c for c, _ in leftover0) if p == 0 else 0
                qk_done = 0
            pt_t[gi0] = ptpool.tile([128, TT, S], bf16, tag="pt", name=f"pt{gi0}")
            pt_t[gi1] = ptpool.tile([128, TT, S], bf16, tag="pt", name=f"pt{gi1}")

            for tt in range(TT):
                slot = p * TT + tt
                tsl = slice(tt * 128, (tt + 1) * 128)
                # exp(bias) prefetch: 2 tiles per slot, shallow at first
                # shallow at first so startup HBM bandwidth goes to xt
                emit_eb_upto(2 * slot + (2 if slot < 2 else 10))

                # row-tiled scores: head h contracts over its own 64
                # features on PE row-half h; the two heads' matmuls run
                # concurrently on disjoint row groups into separate psum
                # tiles (auto tile_position from base_partition).
                psA = ps_scores.tile([128, S], f32, tag="ps_scores", name="psA")
                psB = ps_scores.tile([128, S], f32, tag="ps_scores", name="psB")
                for sc in range(SC):
                    ssl = slice(sc * 512, (sc + 1) * 512)
                    nc.tensor.matmul(
                        psA[:, ssl],
                        kt_t[p][0:HD, tsl],
                        qt_t[p][0:HD, ssl],
                        start=True,
                        stop=True,
                    )
                    nc.tensor.matmul(
                        psB[:, ssl],
                        kt_t[p][HD:128, tsl],
                        qt_t[p][HD:128, ssl],
                        start=True,
                        stop=True,
                    )
                # exp on scalar engine (PSUM -> SBUF bf16), then fold in
                # exp(bias) on vector engine (bf16 x bf16, in-place)
                for h, psx, gi in ((0, psA, gi0), (1, psB, gi1)):
                    nc.scalar.activation(
                        out=pt_t[gi][:, tt, :], in_=psx[:], func=Exp
                    )
                    nc.vector.tensor_tensor(
                        out=pt_t[gi][:, tt, :],
                        in0=pt_t[gi][:, tt, :],
                        in1=eb_tiles.pop((gi, tt))[:],
                        op=Mult,
                    )

                # PV chunks for the previous pair's two heads
                if p >= 1:
                    emit_pv_chunk(gi0 - 2, tt)
                    emit_pv_chunk(gi1 - 2, tt)

                # open the next v stream when its window begins: v(pv)
                # spans [8*pv - 4, 8*pv + 4); v(0) spans [0, 4).
                if v_gen is None and v_next < NP:
                    ws = max(0, 8 * v_next - 4)
                    if slot >= ws:
                        ops = list(gen_v(v_next))
                        v_gen = iter(ops)
                        v_total = sum(c for c, _ in ops)
                        v_done = 0
                        v_start = ws
                        v_next += 1

                # interleaved projection ops: qk stream (pair-aligned) and
                # v stream (offset by 4 slots)
                if qk_gen is not None:
                    budget = (tt + 1) * qk_total / TT
                    while qk_done < budget:
                        try:
                            cols, fn = next(qk_gen)
                        except StopIteration:
                            qk_gen = None
                            break
                        fn()
                        qk_done += cols
                if v_gen is not None:
                    wlen = 4 if v_next == 1 else 8
                    budget = (slot - v_start + 1) * v_total / wlen
                    while v_done < budget:
                        try:
                            cols, fn = next(v_gen)
                        except StopIteration:
                            v_gen = None
                            break
                        fn()
                        v_done += cols

        # tail: PV for the last pair
        for sc8 in range(TT):
            emit_pv_chunk(NHEADS - 2, sc8)
            emit_pv_chunk(NHEADS - 1, sc8)

    nc.compile()
    return nc


def kernel(x, attn_bias, attn_mask, padding_mask, Wq, Wk, Wv):
    global LAST_RESULTS
    from concourse.bass_utils import run_bass_kernel_spmd

    x = np.asarray(x, dtype=np.float32)
    attn_bias = np.asarray(attn_bias, dtype=np.float32)
    attn_mask = np.asarray(attn_mask)
    padding_mask = np.asarray(padding_mask)
    Wq = np.asarray(Wq, dtype=np.float32)
    Wk = np.asarray(Wk, dtype=np.float32)
    Wv = np.asarray(Wv, dtype=np.float32)

    scaling = HD ** -0.5
    # x^T per batch, partition-interleaved: [B, 128(p), KO, S]
    xt_full = np.ascontiguousarray(
        x.transpose(0, 2, 1).reshape(B, KO, 128, S).transpose(0, 2, 1, 3)
    ).astype(BF16)
    wqT = np.ascontiguousarray((Wq * scaling).T).astype(BF16)  # [k, j_global]
    wkT = np.ascontiguousarray(Wk.T).astype(BF16)
    wvT = np.ascontiguousarray(Wv.T).astype(BF16)

    bias_eff = attn_bias
    if attn_mask.any():
        bias_eff = bias_eff + np.where(attn_mask, -1e30, 0.0).astype(np.float32)[
            None, None
        ]
    if padding_mask.any():
        bias_eff = bias_eff + np.where(padding_mask, -1e30, 0.0).astype(np.float32)[
            :, None, None, :
        ]
    # [B, NH, t, s] so scores come out in transposed layout; exp() on host so
    # the kernel multiplies instead of adds (exp(-1e30) == 0 handles masks).
    expbT = np.exp(bias_eff.transpose(0, 1, 3, 2)).astype(BF16)
    # partition-interleave t: [B, NH, 128(p), TT, S]
    expb_r = np.ascontiguousarray(
        expbT.reshape(B, NUM_HEAD, TT, 128, S).transpose(0, 1, 3, 2, 4)
    )

    def relay_w(w):  # [H(k), 512(j)] -> [128(p), NP, KO, 128(j)]
        return np.ascontiguousarray(
            w.reshape(KO, 128, NP, 128).transpose(1, 2, 0, 3)
        )

    in_maps = []
    for c in range(N_CORES):
        bc, gc = divmod(c, 2)
        jsl = slice(gc * 512, (gc + 1) * 512)
        in_maps.append(
            {
                "xt": xt_full[bc],
                "wq": relay_w(wqT[:, jsl]),
                "wk": relay_w(wkT[:, jsl]),
                "wv": relay_w(wvT[:, jsl]),
                "expb": np.ascontiguousarray(
                    expb_r[bc, gc * 8 : (gc + 1) * 8]
                ),
            }
        )

    if "nc" not in _CACHE:
        _CACHE["nc"] = _build_bass()
    nc = _CACHE["nc"]

    res = run_bass_kernel_spmd(
        nc, in_maps, core_ids=list(range(N_CORES)), trace=TRACE
    )
    LAST_RESULTS = res

    full = np.empty((B, S, H), np.float32)
    for c in range(N_CORES):
        bc, gc = divmod(c, 2)
        oc = np.asarray(res.results[c]["out"]).astype(np.float32)
        oc = oc.reshape(NHEADS, 128, TT, HD + 1)
        num = oc[..., :HD]
        den = oc[..., HD]
        o = num / den[..., None]                        # [nh, p, sc, d]
        o = o.transpose(0, 2, 1, 3).reshape(NHEADS, S, HD)  # s = sc*128 + p
        full[bc, :, gc * 512 : (gc + 1) * 512] = (
            o.transpose(1, 0, 2).reshape(S, NHEADS * HD)
        )
    return full


# revision 24
# speedup vs baseline: 1.1877x; 1.1877x over previous
"""Multi-head attention (B=4, S=1024, H=1024, 16 heads) on 8 TRN2 NeuronCores.

Sharding: batch x head-group - each core owns (one batch, 8 heads) as
cores = 4 batches x 2 head-groups.  x^T for the core's batch is loaded once
(2.1 MB) and stays resident; projections run per head-pair (128 j columns).

Scores are computed in transposed [t, s] layout with ROW-TILED matmuls:
each head contracts over only its own 64 features, so the two heads of a
pair run as two concurrent K=64 matmuls on disjoint PE row halves
(tile_position (0,0) / (64,0)) writing separate PSUM tiles.  This removes
the 2x zero-padding waste of K=128 padded scores and the kt pad memsets.

Softmax bias handling uses exp(s + b) = exp(s) * exp(b): the host
precomputes exp(bias^T) in bf16, the scalar engine computes exp(scores)
straight out of PSUM (its only job), and the vector engine multiplies by
the bias factor as a bf16*bf16 SBUF op (2x DVE mode).  The softmax
denominator rides along as a ones-column appended to V; normalization
happens on the host.

The emission schedule is software-pipelined over 32 pair-slots (pair, tt):
projections for pair p+1 interleave into pair p's attention slots, PV for
pair p-1 runs during pair p's score slots.  Steady state alternates
between ACT (2 exps ~2.1us/slot) and PE as the gate; PE fills its slack
with projection/PV work.

PSUM budget (8 banks): scores 3x[128,1024] (6) - the third buffer means a
slot's score matmuls are gated by exps from 1.5 slots back (long retired),
keeping the ACT pipeline fed and letting the row-tiled head pairs
co-issue; one shared 2-buf pool of one-bank tiles (2) serves q/k chain
accumulators, v tt-groups, and PV accumulator groups of two s-chunks.

All DRAM operands are host-relaid so every DMA packet is a 2-4 KiB
contiguous run per partition.  Compute dtype bf16 (f32 PSUM); masks fold
into the bias on host as -1e30 (exp == 0) if ever set.
"""

import numpy as np
import ml_dtypes

NUM_HEAD = 16
B, S, H = 4, 1024, 1024
HD = H // NUM_HEAD            # 64
N_CORES = 8
KO = H // 128                 # 8 contraction chunks
SC = S // 512                 # 2 (N=512 matmul chunks)
TT = S // 128                 # 8 (128-row tiles over s or t)
NP = 4                        # head-pairs per core
HPC = 2                       # heads per pair
NHEADS = NP * HPC             # 8 heads per core

BF16 = ml_dtypes.bfloat16

_CACHE = {}

# Set by test harness to capture profiling info.
TRACE = False
LAST_RESULTS = None


def _build_bass():
    from concourse import bacc
    import concourse.tile as tile
    import concourse.mybir as mybir
    from contextlib import ExitStack

    bf16 = mybir.dt.bfloat16
    f32 = mybir.dt.float32
    Exp = mybir.ActivationFunctionType.Exp
    Mult = mybir.AluOpType.mult

    nc = bacc.Bacc("TRN2", target_bir_lowering=False, debug=False)

    # host-relaid operands: per-partition lines are contiguous DRAM runs
    xt = nc.dram_tensor("xt", [128, KO, S], bf16, kind="ExternalInput")
    wq = nc.dram_tensor("wq", [128, NP, KO, 128], bf16, kind="ExternalInput")
    wk = nc.dram_tensor("wk", [128, NP, KO, 128], bf16, kind="ExternalInput")
    wv = nc.dram_tensor("wv", [128, NP, KO, 128], bf16, kind="ExternalInput")
    expb = nc.dram_tensor("expb", [NHEADS, 128, TT, S], bf16, kind="ExternalInput")
    out = nc.dram_tensor(
        "out", [NHEADS, 128, TT * (HD + 1)], bf16, kind="ExternalOutput"
    )

    with tile.TileContext(nc) as tc, ExitStack() as ctx:
        singles = ctx.enter_context(tc.tile_pool(name="singles", bufs=1))
        qtpool = ctx.enter_context(tc.tile_pool(name="qt", bufs=2))
        ktpool = ctx.enter_context(tc.tile_pool(name="kt", bufs=2))
        vpool = ctx.enter_context(tc.tile_pool(name="v", bufs=3))
        ptpool = ctx.enter_context(tc.tile_pool(name="pt", bufs=4))
        ebpool = ctx.enter_context(tc.tile_pool(name="eb", bufs=10))
        obhpool = ctx.enter_context(tc.tile_pool(name="obh", bufs=3))
        # shared one-bank pool for q/k chain accumulators, v tt-groups and
        # PV accumulator groups (2 banks total)
        ps_proj = ctx.enter_context(tc.tile_pool(name="ps_proj", bufs=2, space="PSUM"))
        # 3 score buffers (6 banks): a slot's score matmuls are then gated
        # by exps from 1.5 slots back (long retired), so the ACT pipeline
        # never waits on PE and the row-tiled head pairs can co-issue.
        ps_scores = ctx.enter_context(
            tc.tile_pool(name="ps_scores", bufs=3, space="PSUM")
        )

        # PE warm-up: a dead-weight matmul chain on (uninitialized) SBUF
        # starts the moment the PE comes up, covering the p-state ramp while
        # the first DMAs land.  The result is discarded.
        warm = singles.tile([128, 512], bf16, tag="warm")
        wps = ps_proj.tile([128, 512], f32, tag="ps_proj")
        NWARM = 26  # sized to cover the ~10us xt DMA latency at cold clock
        for i in range(NWARM):
            nc.tensor.matmul(
                wps[:], warm[:, 0:128], warm[:], start=(i == 0), stop=(i == NWARM - 1)
            )
        nc.vector.tensor_copy(out=warm[:, 0:1], in_=wps[:, 0:1])

        # Startup-critical loads: pair 0's weights and xt chunks first.
        w_sb = {}
        w_dram = {}
        for name, dram in (("q", wq), ("k", wk), ("v", wv)):
            w_sb[name] = singles.tile(
                [128, NP, KO, 128], bf16, tag=f"w{name}", name=f"w{name}_sb"
            )
            w_dram[name] = dram
        nc.sync.dma_start(out=w_sb["q"][:, 0, :, :], in_=wq[:, 0, :, :])
        nc.sync.dma_start(out=w_sb["k"][:, 0, :, :], in_=wk[:, 0, :, :])

        xt_sb = singles.tile([128, KO, S], bf16, tag="xt", name="xt_sb")
        # chunk-pair loads: 4 KiB contiguous per partition per transfer,
        # split into partition halves for transfer concurrency.
        for kc in range(0, KO, 2):
            nc.sync.dma_start(
                out=xt_sb[0:64, kc : kc + 2, :], in_=xt[0:64, kc : kc + 2, :]
            )
            nc.sync.dma_start(
                out=xt_sb[64:128, kc : kc + 2, :], in_=xt[64:128, kc : kc + 2, :]
            )

        qt_t = {}
        kt_t = {}
        vext_t = {}

        def gen_proj(p):
            """Yield (cols, fn) ops emitting head-pair p's q/k projections.

            Chains allocate one-bank [128,512] psum tiles from the shared
            2-buf pool; sc0/sc1 chains of the same proj pipeline through the
            two buffers, and the psum->sbuf cast rides with the last MM.
            """
            if p > 0:
                def dma_w(p=p):
                    for name in ("q", "k", "v"):
                        nc.sync.dma_start(
                            out=w_sb[name][:, p, :, :],
                            in_=w_dram[name][:, p, :, :],
                        )
                yield (0, dma_w)

            def alloc(p=p):
                qt_t[p] = qtpool.tile([128, S], bf16, tag="qt", name=f"qt{p}")
                kt_t[p] = ktpool.tile([128, S], bf16, tag="kt", name=f"kt{p}")
            yield (0, alloc)

            # q then k: [j on partitions, s free]; per 512-chunk a psum chain
            # of 8 K-accumulation matmuls, cast to bf16 SBUF when done.
            for name in ("q", "k"):
                for sc in range(SC):
                    ssl = slice(sc * 512, (sc + 1) * 512)
                    chain = {}
                    for kop in range(0, KO, 2):
                        def mm_pair(
                            p=p, name=name, ssl=ssl, kop=kop, chain=chain
                        ):
                            if kop == 0:
                                chain["ps"] = ps_proj.tile(
                                    [128, 512], f32,
                                    tag="ps_proj", name="ps_projc",
                                )
                            ps = chain["ps"]
                            for ko in (kop, kop + 1):
                                nc.tensor.matmul(
                                    ps[:],
                                    w_sb[name][:, p, ko, :],
                                    xt_sb[:, ko, ssl],
                                    start=(ko == 0),
                                    stop=(ko == KO - 1),
                                )
                            if kop + 2 == KO:
                                dst = qt_t[p] if name == "q" else kt_t[p]
                                nc.vector.tensor_copy(
                                    out=dst[:, ssl], in_=ps[:]
                                )
                        yield (1024, mm_pair)

        def gen_v(p):
            # v: [t on partitions, d free]; tt-chains grouped 4 per psum
            # bank, one combined cast per group into the vext slots.
            if p == 0:
                def dma_wv0():
                    nc.sync.dma_start(
                        out=w_sb["v"][:, 0, :, :], in_=wv[:, 0, :, :]
                    )
                yield (0, dma_wv0)

            def alloc_v(p=p):
                vext_t[p] = vpool.tile(
                    [128, HPC, TT, HD + 1], bf16, tag="vext", name=f"vext{p}"
                )
                nc.gpsimd.memset(vext_t[p][:, :, :, HD : HD + 1], 1.0)
            yield (0, alloc_v)
            group = {}
            for tt in range(TT):
                tsl = slice(tt * 128, (tt + 1) * 128)
                for kop in range(0, KO, 2):
                    def mm_pair_v(p=p, tsl=tsl, tt=tt, kop=kop, group=group):
                        if tt % 4 == 0 and kop == 0:
                            group["ps"] = ps_proj.tile(
                                [128, 4, HPC, HD], f32,
                                tag="ps_proj", name="ps_projv",
                            )
                        ps = group["ps"]
                        for ko in (kop, kop + 1):
                            nc.tensor.matmul(
                                ps[:, tt % 4, :, :],
                                xt_sb[:, ko, tsl],
                                w_sb["v"][:, p, ko, :],
                                start=(ko == 0),
                                stop=(ko == KO - 1),
                            )
                        if kop + 2 == KO and tt % 4 == 3:
                            # cast the 4-tt group; AP traversal order matches
                            # psum (tt, hpc, d) via strided out dims
                            g0 = tt - 3
                            nc.vector.tensor_copy(
                                out=vext_t[p][:, :, g0 : g0 + 4, 0:HD]
                                .transpose([0, 2, 1, 3]),
                                in_=ps[:, :, :, :],
                            )
                    yield (256, mm_pair_v)

        # Drain upfront what the first score tiles need.  q-sc0 and k-sc0
        # are interleaved chunk-wise (both advance while xt streams in) so
        # the first score matmuls (which need only the sc0 halves of qt/kt)
        # unblock one full chain earlier; q-sc1 (needed by the first exp)
        # follows over the already-resident chunks.
        ops0 = list(gen_proj(0))
        # ops0: [alloc, q-sc0 x4, q-sc1 x4, k-sc0 x4, k-sc1 x4]
        order = [ops0[0]]
        for i in range(4):
            order.append(ops0[1 + i])
            order.append(ops0[9 + i])
        order.extend(ops0[5:9])
        for _cols, fn in order:
            fn()
        leftover0 = ops0[13:]

        TOTAL_SLOTS = NP * TT  # 32 pair-slots
        eb_tiles = {}
        eb_cursor = 0

        def emit_eb_upto(limit):
            # eb tile index order: (pair, tt, h) = consumption order
            nonlocal eb_cursor
            while eb_cursor < min(limit, TOTAL_SLOTS * HPC):
                ps_, rem = divmod(eb_cursor, TT * HPC)
                tt2, h2 = divmod(rem, HPC)
                gi2 = ps_ * HPC + h2
                ebt = ebpool.tile([128, S], bf16, tag="eb", name=f"eb{eb_cursor}")
                nc.sync.dma_start(out=ebt[:], in_=expb[gi2, :, tt2, :])
                eb_tiles[(gi2, tt2)] = ebt
                eb_cursor += 1

        pt_t = {}
        obh_t = {}
        pso_cur = {}

        def emit_pv_chunk(gi_prev, sc8):
            hp = gi_prev % HPC
            if sc8 == 0:
                obh_t[gi_prev] = obhpool.tile(
                    [128, TT, HD + 1], bf16, tag="obh", name=f"obh{gi_prev}"
                )
            if hp == 0 and sc8 % 2 == 0:
                pso_cur[0] = ps_proj.tile(
                    [128, HPC, 2, HD + 1], f32, tag="ps_proj", name="pso_g"
                )
            pso = pso_cur[0]
            ssl = slice(sc8 * 128, (sc8 + 1) * 128)
            ptp = pt_t[gi_prev]
            vxp = vext_t[gi_prev // HPC]
            for ttp in range(TT):
                nc.tensor.matmul(
                    pso[:, hp, sc8 % 2, :],
                    ptp[:, ttp, ssl],
                    vxp[:, hp, ttp, :],
                    start=(ttp == 0),
                    stop=(ttp == TT - 1),
                )
            if sc8 % 2 == 1:
                nc.vector.tensor_copy(
                    out=obh_t[gi_prev][:, sc8 - 1 : sc8 + 1, :],
                    in_=pso[:, hp, :, :],
                )
                if sc8 % 4 == 3:
                    half = slice((sc8 - 3) * (HD + 1), (sc8 + 1) * (HD + 1))
                    nc.sync.dma_start(
                        out=out[gi_prev][:, half],
                        in_=obh_t[gi_prev][:, sc8 - 3 : sc8 + 1, :],
                    )

        qk_gen = None
        qk_total = qk_done = 0
        v_gen = None
        v_total = v_done = 0
        v_start = 0
        v_next = 0

        for p in range(NP):
            gi0, gi1 = p * HPC, p * HPC + 1
            # qk stream for pair p+1 over this pair's 8 slots (pair 0's
            # leftover k-sc1 rides at the front of pair 0's window)
            if p + 1 < NP:
                ops = (leftover0 if p == 0 else []) + list(gen_proj(p + 1))
                qk_gen = iter(ops)
                qk_total = sum(c for c, _ in ops)
                qk_done = 0
            else:
                qk_gen = iter(leftover0) if p == 0 else None
                qk_total = sum(c for c, _ in leftover0) if p == 0 else 0
                qk_done = 0
            pt_t[gi0] = ptpool.tile([128, TT, S], bf16, tag="pt", name=f"pt{gi0}")
            pt_t[gi1] = ptpool.tile([128, TT, S], bf16, tag="pt", name=f"pt{gi1}")

            for tt in range(TT):
                slot = p * TT + tt
                tsl = slice(tt * 128, (tt + 1) * 128)
                # exp(bias) prefetch: 2 tiles per slot, shallow at first
                # shallow at first so startup HBM bandwidth goes to xt
                emit_eb_upto(2 * slot + (2 if slot < 2 else 10))

                # row-tiled scores: head h contracts over its own 64
                # features on PE row-half h; the two heads' matmuls run
                # concurrently on disjoint row groups into separate psum
                # tiles (auto tile_position from base_partition).
                psA = ps_scores.tile([128, S], f32, tag="ps_scores", name="psA")
                psB = ps_scores.tile([128, S], f32, tag="ps_scores", name="psB")
                for sc in range(SC):
                    ssl = slice(sc * 512, (sc + 1) * 512)
                    nc.tensor.matmul(
                        psA[:, ssl],
                        kt_t[p][0:HD, tsl],
                        qt_t[p][0:HD, ssl],
                        start=True,
                        stop=True,
                    )
                    nc.tensor.matmul(
                        psB[:, ssl],
                        kt_t[p][HD:128, tsl],
                        qt_t[p][HD:128, ssl],
                        start=True,
                        stop=True,
                    )
                # exp on scalar engine (PSUM -> SBUF bf16), then fold in
                # exp(bias) on vector engine (bf16 x bf16, in-place)
                for h, psx, gi in ((0, psA, gi0), (1, psB, gi1)):
                    nc.scalar.activation(
                        out=pt_t[gi][:, tt, :], in_=psx[:], func=Exp
                    )
                    nc.vector.tensor_tensor(
                        out=pt_t[gi][:, tt, :],
                        in0=pt_t[gi][:, tt, :],
                        in1=eb_tiles.pop((gi, tt))[:],
                        op=Mult,
                    )

                # PV chunks for the previous pair's two heads
                if p >= 1:
                    emit_pv_chunk(gi0 - 2, tt)
                    emit_pv_chunk(gi1 - 2, tt)

                # open the next v stream when its window begins: v(pv)
                # spans [8*pv - 4, 8*pv + 4); v(0) spans [0, 4).
                if v_gen is None and v_next < NP:
                    ws = max(0, 8 * v_next - 4)
                    if slot >= ws:
                        ops = list(gen_v(v_next))
                        v_gen = iter(ops)
                        v_total = sum(c for c, _ in ops)
                        v_done = 0
                        v_start = ws
                        v_next += 1

                # interleaved projection ops: qk stream (pair-aligned) and
                # v stream (offset by 4 slots)
                if qk_gen is not None:
                    budget = (tt + 1) * qk_total / TT
                    while qk_done < budget:
                        try:
                            cols, fn = next(qk_gen)
                        except StopIteration:
                            qk_gen = None
                            break
                        fn()
                        qk_done += cols
                if v_gen is not None:
                    wlen = 4 if v_next == 1 else 8
                    budget = (slot - v_start + 1) * v_total / wlen
                    while v_done < budget:
                        try:
                            cols, fn = next(v_gen)
                        except StopIteration:
                            v_gen = None
                            break
                        fn()
                        v_done += cols

        # tail: PV for the last pair
        for sc8 in range(TT):
            emit_pv_chunk(NHEADS - 2, sc8)
            emit_pv_chunk(NHEADS - 1, sc8)

    nc.compile()
    return nc


def kernel(x, attn_bias, attn_mask, padding_mask, Wq, Wk, Wv):
    global LAST_RESULTS
    from concourse.bass_utils import run_bass_kernel_spmd

    x = np.asarray(x, dtype=np.float32)
    attn_bias = np.asarray(attn_bias, dtype=np.float32)
    attn_mask = np.asarray(attn_mask)
    padding_mask = np.asarray(padding_mask)
    Wq = np.asarray(Wq, dtype=np.float32)
    Wk = np.asarray(Wk, dtype=np.float32)
    Wv = np.asarray(Wv, dtype=np.float32)

    scaling = HD ** -0.5
    # x^T per batch, partition-interleaved: [B, 128(p), KO, S]
    xt_full = np.ascontiguousarray(
        x.transpose(0, 2, 1).reshape(B, KO, 128, S).transpose(0, 2, 1, 3)
    ).astype(BF16)
    wqT = np.ascontiguousarray((Wq * scaling).T).astype(BF16)  # [k, j_global]
    wkT = np.ascontiguousarray(Wk.T).astype(BF16)
    wvT = np.ascontiguousarray(Wv.T).astype(BF16)

    bias_eff = attn_bias
    if attn_mask.any():
        bias_eff = bias_eff + np.where(attn_mask, -1e30, 0.0).astype(np.float32)[
            None, None
        ]
    if padding_mask.any():
        bias_eff = bias_eff + np.where(padding_mask, -1e30, 0.0).astype(np.float32)[
            :, None, None, :
        ]
    # [B, NH, t, s] so scores come out in transposed layout; exp() on host so
    # the kernel multiplies instead of adds (exp(-1e30) == 0 handles masks).
    expbT = np.exp(bias_eff.transpose(0, 1, 3, 2)).astype(BF16)
    # partition-interleave t: [B, NH, 128(p), TT, S]
    expb_r = np.ascontiguousarray(
        expbT.reshape(B, NUM_HEAD, TT, 128, S).transpose(0, 1, 3, 2, 4)
    )

    def relay_w(w):  # [H(k), 512(j)] -> [128(p), NP, KO, 128(j)]
        return np.ascontiguousarray(
            w.reshape(KO, 128, NP, 128).transpose(1, 2, 0, 3)
        )

    in_maps = []
    for c in range(N_CORES):
        bc, gc = divmod(c, 2)
        jsl = slice(gc * 512, (gc + 1) * 512)
        in_maps.append(
            {
                "xt": xt_full[bc],
                "wq": relay_w(wqT[:, jsl]),
                "wk": relay_w(wkT[:, jsl]),
                "wv": relay_w(wvT[:, jsl]),
                "expb": np.ascontiguousarray(
                    expb_r[bc, gc * 8 : (gc + 1) * 8]
                ),
            }
        )

    if "nc" not in _CACHE:
        _CACHE["nc"] = _build_bass()
    nc = _CACHE["nc"]

    res = run_bass_kernel_spmd(
        nc, in_maps, core_ids=list(range(N_CORES)), trace=TRACE
    )
    LAST_RESULTS = res

    full = np.empty((B, S, H), np.float32)
    for c in range(N_CORES):
        bc, gc = divmod(c, 2)
        oc = np.asarray(res.results[c]["out"]).astype(np.float32)
        oc = oc.reshape(NHEADS, 128, TT, HD + 1)
        num = oc[..., :HD]
        den = oc[..., HD]
        o = num / den[..., None]                        # [nh, p, sc, d]
        o = o.transpose(0, 2, 1, 3).reshape(NHEADS, S, HD)  # s = sc*128 + p
        full[bc, :, gc * 512 : (gc + 1) * 512] = (
            o.transpose(1, 0, 2).reshape(S, NHEADS * HD)
        )
    return full


# revision 25
# speedup vs baseline: 1.2124x; 1.0208x over previous
"""Multi-head attention (B=4, S=1024, H=1024, 16 heads) on 8 TRN2 NeuronCores.

Sharding: batch x head-group - each core owns (one batch, 8 heads) as
cores = 4 batches x 2 head-groups.  x^T for the core's batch is loaded once
(2.1 MB) and stays resident; projections run per head-pair (128 j columns).

Scores are computed in transposed [t, s] layout with ROW-TILED matmuls:
each head contracts over only its own 64 features, so the two heads of a
pair run as two concurrent K=64 matmuls on disjoint PE row halves
(tile_position (0,0) / (64,0)) writing separate PSUM tiles.  This removes
the 2x zero-padding waste of K=128 padded scores and the kt pad memsets.

Softmax bias handling uses exp(s + b) = exp(s) * exp(b): the host
precomputes exp(bias^T) in bf16, the scalar engine computes exp(scores)
straight out of PSUM (its only job), and the vector engine multiplies by
the bias factor as a bf16*bf16 SBUF op (2x DVE mode).  The softmax
denominator rides along as a ones-column appended to V; normalization
happens on the host.

The emission schedule is software-pipelined over 32 pair-slots (pair, tt):
projections for pair p+1 interleave into pair p's attention slots, PV for
pair p-1 runs during pair p's score slots.  Steady state alternates
between ACT (2 exps ~2.1us/slot) and PE as the gate; PE fills its slack
with projection/PV work.

PSUM budget (8 banks): scores 3x[128,1024] (6) - the third buffer means a
slot's score matmuls are gated by exps from 1.5 slots back (long retired),
keeping the ACT pipeline fed and letting the row-tiled head pairs
co-issue; one shared 2-buf pool of one-bank tiles (2) serves q/k chain
accumulators, v tt-groups, and PV accumulator groups of two s-chunks.

All DRAM operands are host-relaid so every DMA packet is a 2-4 KiB
contiguous run per partition.  Compute dtype bf16 (f32 PSUM); masks fold
into the bias on host as -1e30 (exp == 0) if ever set.
"""

import numpy as np
import ml_dtypes

NUM_HEAD = 16
B, S, H = 4, 1024, 1024
HD = H // NUM_HEAD            # 64
N_CORES = 8
KO = H // 128                 # 8 contraction chunks
SC = S // 512                 # 2 (N=512 matmul chunks)
TT = S // 128                 # 8 (128-row tiles over s or t)
NP = 4                        # head-pairs per core
HPC = 2                       # heads per pair
NHEADS = NP * HPC             # 8 heads per core

BF16 = ml_dtypes.bfloat16

_CACHE = {}

# Set by test harness to capture profiling info.
TRACE = False
LAST_RESULTS = None


def _build_bass():
    from concourse import bacc
    import concourse.tile as tile
    import concourse.mybir as mybir
    from contextlib import ExitStack

    bf16 = mybir.dt.bfloat16
    f32 = mybir.dt.float32
    Exp = mybir.ActivationFunctionType.Exp
    Mult = mybir.AluOpType.mult

    nc = bacc.Bacc("TRN2", target_bir_lowering=False, debug=False)

    # host-relaid operands: per-partition lines are contiguous DRAM runs
    xt = nc.dram_tensor("xt", [128, KO, S], bf16, kind="ExternalInput")
    wq = nc.dram_tensor("wq", [128, NP, KO, 128], bf16, kind="ExternalInput")
    wk = nc.dram_tensor("wk", [128, NP, KO, 128], bf16, kind="ExternalInput")
    wv = nc.dram_tensor("wv", [128, NP, KO, 128], bf16, kind="ExternalInput")
    expb = nc.dram_tensor("expb", [NHEADS, 128, TT, S], bf16, kind="ExternalInput")
    out = nc.dram_tensor(
        "out", [NHEADS, 128, TT * (HD + 1)], bf16, kind="ExternalOutput"
    )

    with tile.TileContext(nc) as tc, ExitStack() as ctx:
        singles = ctx.enter_context(tc.tile_pool(name="singles", bufs=1))
        qtpool = ctx.enter_context(tc.tile_pool(name="qt", bufs=2))
        ktpool = ctx.enter_context(tc.tile_pool(name="kt", bufs=2))
        vpool = ctx.enter_context(tc.tile_pool(name="v", bufs=3))
        ptpool = ctx.enter_context(tc.tile_pool(name="pt", bufs=4))
        ebpool = ctx.enter_context(tc.tile_pool(name="eb", bufs=10))
        obhpool = ctx.enter_context(tc.tile_pool(name="obh", bufs=3))
        # shared one-bank pool for q/k chain accumulators, v tt-groups and
        # PV accumulator groups (2 banks total)
        ps_proj = ctx.enter_context(tc.tile_pool(name="ps_proj", bufs=2, space="PSUM"))
        # 3 score buffers (6 banks): a slot's score matmuls are then gated
        # by exps from 1.5 slots back (long retired), so the ACT pipeline
        # never waits on PE and the row-tiled head pairs can co-issue.
        ps_scores = ctx.enter_context(
            tc.tile_pool(name="ps_scores", bufs=3, space="PSUM")
        )

        # Hoist the ~1.4us ACT_TABLE_LOAD (lazily inserted before the first
        # Exp) into the startup DMA-wait window: a dummy exp on a small
        # zeroed tile, dependency-free, so the table is resident long
        # before the first real exp on the critical path.
        dum = singles.tile([1, 8], bf16, tag="dum")
        nc.gpsimd.memset(dum[:], 0.0)
        nc.scalar.activation(out=dum[:], in_=dum[:], func=Exp)

        # PE warm-up: a dead-weight matmul chain on (uninitialized) SBUF
        # starts the moment the PE comes up, covering the p-state ramp while
        # the first DMAs land.  The result is discarded.
        warm = singles.tile([128, 512], bf16, tag="warm")
        wps = ps_proj.tile([128, 512], f32, tag="ps_proj")
        NWARM = 26  # sized to cover the ~10us xt DMA latency at cold clock
        for i in range(NWARM):
            nc.tensor.matmul(
                wps[:], warm[:, 0:128], warm[:], start=(i == 0), stop=(i == NWARM - 1)
            )
        nc.vector.tensor_copy(out=warm[:, 0:1], in_=wps[:, 0:1])

        # Startup-critical loads: pair 0's weights and xt chunks first.
        w_sb = {}
        w_dram = {}
        for name, dram in (("q", wq), ("k", wk), ("v", wv)):
            w_sb[name] = singles.tile(
                [128, NP, KO, 128], bf16, tag=f"w{name}", name=f"w{name}_sb"
            )
            w_dram[name] = dram
        nc.sync.dma_start(out=w_sb["q"][:, 0, :, :], in_=wq[:, 0, :, :])
        nc.sync.dma_start(out=w_sb["k"][:, 0, :, :], in_=wk[:, 0, :, :])

        xt_sb = singles.tile([128, KO, S], bf16, tag="xt", name="xt_sb")
        # chunk-pair loads: 4 KiB contiguous per partition per transfer,
        # split into partition halves for transfer concurrency.
        for kc in range(0, KO, 2):
            nc.sync.dma_start(
                out=xt_sb[0:64, kc : kc + 2, :], in_=xt[0:64, kc : kc + 2, :]
            )
            nc.sync.dma_start(
                out=xt_sb[64:128, kc : kc + 2, :], in_=xt[64:128, kc : kc + 2, :]
            )

        qt_t = {}
        kt_t = {}
        vext_t = {}

        def gen_proj(p):
            """Yield (cols, fn) ops emitting head-pair p's q/k projections.

            Chains allocate one-bank [128,512] psum tiles from the shared
            2-buf pool; sc0/sc1 chains of the same proj pipeline through the
            two buffers, and the psum->sbuf cast rides with the last MM.
            """
            if p > 0:
                def dma_w(p=p):
                    for name in ("q", "k", "v"):
                        nc.sync.dma_start(
                            out=w_sb[name][:, p, :, :],
                            in_=w_dram[name][:, p, :, :],
                        )
                yield (0, dma_w)

            def alloc(p=p):
                qt_t[p] = qtpool.tile([128, S], bf16, tag="qt", name=f"qt{p}")
                kt_t[p] = ktpool.tile([128, S], bf16, tag="kt", name=f"kt{p}")
            yield (0, alloc)

            # q then k: [j on partitions, s free]; per 512-chunk a psum chain
            # of 8 K-accumulation matmuls, cast to bf16 SBUF when done.
            for name in ("q", "k"):
                for sc in range(SC):
                    ssl = slice(sc * 512, (sc + 1) * 512)
                    chain = {}
                    for kop in range(0, KO, 2):
                        def mm_pair(
                            p=p, name=name, ssl=ssl, kop=kop, chain=chain
                        ):
                            if kop == 0:
                                chain["ps"] = ps_proj.tile(
                                    [128, 512], f32,
                                    tag="ps_proj", name="ps_projc",
                                )
                            ps = chain["ps"]
                            for ko in (kop, kop + 1):
                                nc.tensor.matmul(
                                    ps[:],
                                    w_sb[name][:, p, ko, :],
                                    xt_sb[:, ko, ssl],
                                    start=(ko == 0),
                                    stop=(ko == KO - 1),
                                )
                            if kop + 2 == KO:
                                dst = qt_t[p] if name == "q" else kt_t[p]
                                nc.vector.tensor_copy(
                                    out=dst[:, ssl], in_=ps[:]
                                )
                        yield (1024, mm_pair)

        def gen_v(p):
            # v: [t on partitions, d free]; tt-chains grouped 4 per psum
            # bank, one combined cast per group into the vext slots.
            if p == 0:
                def dma_wv0():
                    nc.sync.dma_start(
                        out=w_sb["v"][:, 0, :, :], in_=wv[:, 0, :, :]
                    )
                yield (0, dma_wv0)

            def alloc_v(p=p):
                vext_t[p] = vpool.tile(
                    [128, HPC, TT, HD + 1], bf16, tag="vext", name=f"vext{p}"
                )
                nc.gpsimd.memset(vext_t[p][:, :, :, HD : HD + 1], 1.0)
            yield (0, alloc_v)
            group = {}
            for tt in range(TT):
                tsl = slice(tt * 128, (tt + 1) * 128)
                for kop in range(0, KO, 2):
                    def mm_pair_v(p=p, tsl=tsl, tt=tt, kop=kop, group=group):
                        if tt % 4 == 0 and kop == 0:
                            group["ps"] = ps_proj.tile(
                                [128, 4, HPC, HD], f32,
                                tag="ps_proj", name="ps_projv",
                            )
                        ps = group["ps"]
                        for ko in (kop, kop + 1):
                            nc.tensor.matmul(
                                ps[:, tt % 4, :, :],
                                xt_sb[:, ko, tsl],
                                w_sb["v"][:, p, ko, :],
                                start=(ko == 0),
                                stop=(ko == KO - 1),
                            )
                        if kop + 2 == KO and tt % 4 == 3:
                            # cast the 4-tt group; AP traversal order matches
                            # psum (tt, hpc, d) via strided out dims
                            g0 = tt - 3
                            nc.vector.tensor_copy(
                                out=vext_t[p][:, :, g0 : g0 + 4, 0:HD]
                                .transpose([0, 2, 1, 3]),
                                in_=ps[:, :, :, :],
                            )
                    yield (256, mm_pair_v)

        # Drain upfront what the first score tiles need.  q-sc0 and k-sc0
        # are interleaved chunk-wise (both advance while xt streams in) so
        # the first score matmuls (which need only the sc0 halves of qt/kt)
        # unblock one full chain earlier; q-sc1 (needed by the first exp)
        # follows over the already-resident chunks.
        ops0 = list(gen_proj(0))
        # ops0: [alloc, q-sc0 x4, q-sc1 x4, k-sc0 x4, k-sc1 x4]
        order = [ops0[0]]
        for i in range(4):
            order.append(ops0[1 + i])
            order.append(ops0[9 + i])
        order.extend(ops0[5:9])
        for _cols, fn in order:
            fn()
        leftover0 = ops0[13:]

        TOTAL_SLOTS = NP * TT  # 32 pair-slots
        eb_tiles = {}
        eb_cursor = 0

        def emit_eb_upto(limit):
            # eb tile index order: (pair, tt, h) = consumption order
            nonlocal eb_cursor
            while eb_cursor < min(limit, TOTAL_SLOTS * HPC):
                ps_, rem = divmod(eb_cursor, TT * HPC)
                tt2, h2 = divmod(rem, HPC)
                gi2 = ps_ * HPC + h2
                ebt = ebpool.tile([128, S], bf16, tag="eb", name=f"eb{eb_cursor}")
                nc.sync.dma_start(out=ebt[:], in_=expb[gi2, :, tt2, :])
                eb_tiles[(gi2, tt2)] = ebt
                eb_cursor += 1

        pt_t = {}
        obh_t = {}
        pso_cur = {}

        def emit_pv_chunk(gi_prev, sc8):
            hp = gi_prev % HPC
            if sc8 == 0:
                obh_t[gi_prev] = obhpool.tile(
                    [128, TT, HD + 1], bf16, tag="obh", name=f"obh{gi_prev}"
                )
            if hp == 0 and sc8 % 2 == 0:
                pso_cur[0] = ps_proj.tile(
                    [128, HPC, 2, HD + 1], f32, tag="ps_proj", name="pso_g"
                )
            pso = pso_cur[0]
            ssl = slice(sc8 * 128, (sc8 + 1) * 128)
            ptp = pt_t[gi_prev]
            vxp = vext_t[gi_prev // HPC]
            for ttp in range(TT):
                nc.tensor.matmul(
                    pso[:, hp, sc8 % 2, :],
                    ptp[:, ttp, ssl],
                    vxp[:, hp, ttp, :],
                    start=(ttp == 0),
                    stop=(ttp == TT - 1),
                )
            if sc8 % 2 == 1:
                nc.vector.tensor_copy(
                    out=obh_t[gi_prev][:, sc8 - 1 : sc8 + 1, :],
                    in_=pso[:, hp, :, :],
                )
                if sc8 % 4 == 3:
                    half = slice((sc8 - 3) * (HD + 1), (sc8 + 1) * (HD + 1))
                    nc.sync.dma_start(
                        out=out[gi_prev][:, half],
                        in_=obh_t[gi_prev][:, sc8 - 3 : sc8 + 1, :],
                    )

        qk_gen = None
        qk_total = qk_done = 0
        v_gen = None
        v_total = v_done = 0
        v_start = 0
        v_next = 0

        for p in range(NP):
            gi0, gi1 = p * HPC, p * HPC + 1
            # qk stream for pair p+1 over this pair's 8 slots (pair 0's
            # leftover k-sc1 rides at the front of pair 0's window)
            if p + 1 < NP:
                ops = (leftover0 if p == 0 else []) + list(gen_proj(p + 1))
                qk_gen = iter(ops)
                qk_total = sum(c for c, _ in ops)
                qk_done = 0
            else:
                qk_gen = iter(leftover0) if p == 0 else None
                qk_total = sum(c for c, _ in leftover0) if p == 0 else 0
                qk_done = 0
            pt_t[gi0] = ptpool.tile([128, TT, S], bf16, tag="pt", name=f"pt{gi0}")
            pt_t[gi1] = ptpool.tile([128, TT, S], bf16, tag="pt", name=f"pt{gi1}")

            for tt in range(TT):
                slot = p * TT + tt
                tsl = slice(tt * 128, (tt + 1) * 128)
                # exp(bias) prefetch: 2 tiles per slot, shallow at first
                # shallow at first so startup HBM bandwidth goes to xt
                emit_eb_upto(2 * slot + (2 if slot < 2 else 10))

                # row-tiled scores: head h contracts over its own 64
                # features on PE row-half h; the two heads' matmuls run
                # concurrently on disjoint row groups into separate psum
                # tiles (auto tile_position from base_partition).
                psA = ps_scores.tile([128, S], f32, tag="ps_scores", name="psA")
                psB = ps_scores.tile([128, S], f32, tag="ps_scores", name="psB")
                for sc in range(SC):
                    ssl = slice(sc * 512, (sc + 1) * 512)
                    nc.tensor.matmul(
                        psA[:, ssl],
                        kt_t[p][0:HD, tsl],
                        qt_t[p][0:HD, ssl],
                        start=True,
                        stop=True,
                    )
                    nc.tensor.matmul(
                        psB[:, ssl],
                        kt_t[p][HD:128, tsl],
                        qt_t[p][HD:128, ssl],
                        start=True,
                        stop=True,
                    )
                # exp on scalar engine (PSUM -> SBUF bf16), then fold in
                # exp(bias) on vector engine (bf16 x bf16, in-place)
                for h, psx, gi in ((0, psA, gi0), (1, psB, gi1)):
                    nc.scalar.activation(
                        out=pt_t[gi][:, tt, :], in_=psx[:], func=Exp
                    )
                    nc.vector.tensor_tensor(
                        out=pt_t[gi][:, tt, :],
                        in0=pt_t[gi][:, tt, :],
                        in1=eb_tiles.pop((gi, tt))[:],
                        op=Mult,
                    )

                # PV chunks for the previous pair's two heads
                if p >= 1:
                    emit_pv_chunk(gi0 - 2, tt)
                    emit_pv_chunk(gi1 - 2, tt)

                # open the next v stream when its window begins: v(pv)
                # spans [8*pv - 4, 8*pv + 4); v(0) spans [0, 4).
                if v_gen is None and v_next < NP:
                    ws = max(0, 8 * v_next - 4)
                    if slot >= ws:
                        ops = list(gen_v(v_next))
                        v_gen = iter(ops)
                        v_total = sum(c for c, _ in ops)
                        v_done = 0
                        v_start = ws
                        v_next += 1

                # interleaved projection ops: qk stream (pair-aligned) and
                # v stream (offset by 4 slots)
                if qk_gen is not None:
                    budget = (tt + 1) * qk_total / TT
                    while qk_done < budget:
                        try:
                            cols, fn = next(qk_gen)
                        except StopIteration:
                            qk_gen = None
                            break
                        fn()
                        qk_done += cols
                if v_gen is not None:
                    wlen = 4 if v_next == 1 else 8
                    budget = (slot - v_start + 1) * v_total / wlen
                    while v_done < budget:
                        try:
                            cols, fn = next(v_gen)
                        except StopIteration:
                            v_gen = None
                            break
                        fn()
                        v_done += cols

        # tail: PV for the last pair
        for sc8 in range(TT):
            emit_pv_chunk(NHEADS - 2, sc8)
            emit_pv_chunk(NHEADS - 1, sc8)

    nc.compile()
    return nc


def kernel(x, attn_bias, attn_mask, padding_mask, Wq, Wk, Wv):
    global LAST_RESULTS
    from concourse.bass_utils import run_bass_kernel_spmd

    x = np.asarray(x, dtype=np.float32)
    attn_bias = np.asarray(attn_bias, dtype=np.float32)
    attn_mask = np.asarray(attn_mask)
    padding_mask = np.asarray(padding_mask)
    Wq = np.asarray(Wq, dtype=np.float32)
    Wk = np.asarray(Wk, dtype=np.float32)
    Wv = np.asarray(Wv, dtype=np.float32)

    scaling = HD ** -0.5
    # x^T per batch, partition-interleaved: [B, 128(p), KO, S]
    xt_full = np.ascontiguousarray(
        x.transpose(0, 2, 1).reshape(B, KO, 128, S).transpose(0, 2, 1, 3)
    ).astype(BF16)
    wqT = np.ascontiguousarray((Wq * scaling).T).astype(BF16)  # [k, j_global]
    wkT = np.ascontiguousarray(Wk.T).astype(BF16)
    wvT = np.ascontiguousarray(Wv.T).astype(BF16)

    bias_eff = attn_bias
    if attn_mask.any():
        bias_eff = bias_eff + np.where(attn_mask, -1e30, 0.0).astype(np.float32)[
            None, None
        ]
    if padding_mask.any():
        bias_eff = bias_eff + np.where(padding_mask, -1e30, 0.0).astype(np.float32)[
            :, None, None, :
        ]
    # [B, NH, t, s] so scores come out in transposed layout; exp() on host so
    # the kernel multiplies instead of adds (exp(-1e30) == 0 handles masks).
    expbT = np.exp(bias_eff.transpose(0, 1, 3, 2)).astype(BF16)
    # partition-interleave t: [B, NH, 128(p), TT, S]
    expb_r = np.ascontiguousarray(
        expbT.reshape(B, NUM_HEAD, TT, 128, S).transpose(0, 1, 3, 2, 4)
    )

    def relay_w(w):  # [H(k), 512(j)] -> [128(p), NP, KO, 128(j)]
        return np.ascontiguousarray(
            w.reshape(KO, 128, NP, 128).transpose(1, 2, 0, 3)
        )

    in_maps = []
    for c in range(N_CORES):
        bc, gc = divmod(c, 2)
        jsl = slice(gc * 512, (gc + 1) * 512)
        in_maps.append(
            {
                "xt": xt_full[bc],
                "wq": relay_w(wqT[:, jsl]),
                "wk": relay_w(wkT[:, jsl]),
                "wv": relay_w(wvT[:, jsl]),
                "expb": np.ascontiguousarray(
                    expb_r[bc, gc * 8 : (gc + 1) * 8]
                ),
            }
        )

    if "nc" not in _CACHE:
        _CACHE["nc"] = _build_bass()
    nc = _CACHE["nc"]

    res = run_bass_kernel_spmd(
        nc, in_maps, core_ids=list(range(N_CORES)), trace=TRACE
    )
    LAST_RESULTS = res

    full = np.empty((B, S, H), np.float32)
    for c in range(N_CORES):
        bc, gc = divmod(c, 2)
        oc = np.asarray(res.results[c]["out"]).astype(np.float32)
        oc = oc.reshape(NHEADS, 128, TT, HD + 1)
        num = oc[..., :HD]
        den = oc[..., HD]
        o = num / den[..., None]                        # [nh, p, sc, d]
        o = o.transpose(0, 2, 1, 3).reshape(NHEADS, S, HD)  # s = sc*128 + p
        full[bc, :, gc * 512 : (gc + 1) * 512] = (
            o.transpose(1, 0, 2).reshape(S, NHEADS * HD)
        )
    return full
